# revision 27
# baseline (speedup 1.0000x reference)
"""Trainium2 Bass kernel for nn_DenoiseNet_28767690949312 (denoising score loss).

Per-core layout (core = 2*b + h): batch b, queries q in [64h, 64h+64).
Structure:
- noisy KNNs (feat k=16, frame k=32) in a dup-half layout [128, 2048]
  with candidate indices embedded in the low 12 mantissa bits (bf16
  6-term split-product distance matmuls, fp32 PSUM accumulation), so
  selection is MAX8 + MATCH_REPLACE rounds, then a cheap [64, 2k] merge.
- clean KNN is pruned: the host ships, per noisy point, a CF-candidate
  clean window (k-d-tree leaf box query around the point — index build +
  range lookup only, no host-side distance sorting). The per-frame-point
  top-4 among its candidates + masked z-sum run on [128, CF] tiles:
  two fused STT ops build -d^2 (expansion form, per-row shift folded),
  MAX8 gives the exact 4th-NN threshold, one masked STT accumulates z.
- score net: per-512-col chunk, residual carry accumulates IN PLACE in
  PSUM (no identity matmul, no net copy); BN/bias constants are folded
  cumulatively on the host.
"""
import contextlib
import sys

import numpy as np

sys.path.insert(0, "/opt/trn_rl_repo")

import concourse.bass as bass
import concourse.mybir as mybir
import concourse.tile as tile

F32 = mybir.dt.float32
I32 = mybir.dt.int32
U16 = mybir.dt.uint16
BF16 = mybir.dt.bfloat16
AF = mybir.ActivationFunctionType
OP = mybir.AluOpType

B, N_NOISY, N_CLEAN = 4, 4096, 4500
N_TRAIN, K_FRAME = 128, 32
DSM_SIGMA = 0.01
FE_HID, FEAT_DIM = 64, 128
HID, NUM_BLOCKS = 128, 4
QH = 64
HALF = 2048
NEG = -1.0e30

# clean-knn candidate window (per noisy point)
CF = 64              # candidates per noisy point
NTAB_W = 4 + 4 * CF  # ntab cols: 0:3 xyz, 3 pad, then tcx|tcy|nc2|cz blocks
KAPPA, ANC, NMIN, LEAF = 2.2, 64, 24, 8
# noisy-knn candidate windows (per query, dup-half layout)
CF3H = 96            # feat (3D, k=16): 192 candidates -> 96 per half
CFNH = 80            # frame (2D, k=32): 160 candidates -> 80 per half


def _split_multi_waits(nc, max_waits=1):
    for func in nc.m.functions:
        for bb in func.blocks:
            il = bb.instructions
            out = []
            changed = False
            for inst in il:
                si = inst.sync_info
                waits = list(si.on_wait) if (si is not None and si.on_wait) else []
                if len(waits) > max_waits:
                    for w in waits[:-max_waits]:
                        ev = mybir.InstEventSemaphore(
                            name=f"I-wsplit-{nc.next_id()}", ins=[], outs=[])
                        ev.engine = inst.engine
                        ev.sync_info = mybir.SyncInfo(on_wait=[w], on_update=[])
                        out.append(ev)
                    inst.sync_info = mybir.SyncInfo(
                        on_wait=waits[-max_waits:],
                        on_update=list(si.on_update) if si.on_update else [])
                    changed = True
                out.append(inst)
            if changed:
                bb.instructions = out


def _bcast_qk(ap_2d, q0, nq, nk):
    sl = ap_2d[:, q0:q0 + nq]
    return bass.AP(sl.tensor, sl.offset, [sl.ap[0], [sl.ap[-1][0], nq], [0, nk]])


def _build():
    nc = bass.Bass()
    noisy = nc.dram_tensor("noisy", [N_NOISY, 3], F32, kind="ExternalInput")
    ntab = nc.dram_tensor("ntab", [N_NOISY, NTAB_W], F32, kind="ExternalInput")
    qtabA_in = nc.dram_tensor("qtabA", [128, 4 * CF3H], F32, kind="ExternalInput")
    gbitsA_in = nc.dram_tensor("gbitsA", [128, CF3H], I32, kind="ExternalInput")
    qtabC_in = nc.dram_tensor("qtabC", [128, 3 * CFNH], F32, kind="ExternalInput")
    gbitsC_in = nc.dram_tensor("gbitsC", [128, CFNH], I32, kind="ExternalInput")
    qscal_in = nc.dram_tensor("qscal", [128, 4], F32, kind="ExternalInput")
    negctrK_in = nc.dram_tensor("negctrK", [4, 48], F32, kind="ExternalInput")
    ind4_in = nc.dram_tensor("ind4", [4, 128], F32, kind="ExternalInput")
    maskc_in = nc.dram_tensor("maskc", [128, 2], I32, kind="ExternalInput")
    ctrTk_in = nc.dram_tensor("ctrTk", [3, QH], F32, kind="ExternalInput")
    ctr2T_in = nc.dram_tensor("ctr2T", [3, 128], F32, kind="ExternalInput")
    ident_in = nc.dram_tensor("ident", [128, 128], F32, kind="ExternalInput")
    # packed weight blobs (host-packed):
    # wbf cols: Wp_f(128) W0[0..3](512) W1[0..3](512) Wc_f[0..3](512) Wo(1)
    wbf_in = nc.dram_tensor("wbf", [128, 1665], U16, kind="ExternalInput")
    # xbf cols: Wp_x(128) Wc_x[0..3](512) feW1b(64); row 3 = bp row (Wp_x only)
    xbf_in = nc.dram_tensor("xbf", [4, 704], U16, kind="ExternalInput")
    # few1c cols: feW1a(64) -feW1b(64), f32 (for the hconst matmul)
    few1c_in = nc.dram_tensor("few1c", [3, 128], F32, kind="ExternalInput")
    few2_in = nc.dram_tensor("few2b", [FE_HID, FEAT_DIM], U16, kind="ExternalInput")
    # bias cols: bp g0x4 hb1x4 g1x4 comb1x4 gO rfb bo feb1 feb2
    bias_in = nc.dram_tensor("biasb", [128, 26], F32, kind="ExternalInput")
    partial = nc.dram_tensor("partial", [1, 1], F32, kind="ExternalOutput")

    with tile.TileContext(nc) as tc, contextlib.ExitStack() as ctx:
        E = ctx.enter_context
        con = E(tc.tile_pool(name="con", bufs=1))
        big = E(tc.tile_pool(name="big", bufs=1))
        sc = E(tc.tile_pool(name="sc", bufs=2))
        dp = E(tc.tile_pool(name="dp", bufs=2))      # MR scratch
        pre = E(tc.tile_pool(name="pre", bufs=16))   # gather prefetch
        ps_net = E(tc.tile_pool(name="psn", bufs=3, space="PSUM"))  # carry
        ps_blk = E(tc.tile_pool(name="psb", bufs=3, space="PSUM"))  # W0 out
        ps_sm = E(tc.tile_pool(name="pss", bufs=2, space="PSUM"))   # small

        # ---------- constant loads (knn-phase inputs first) ----------
        maskc = con.tile([128, 2], I32)
        nc.sync.dma_start(maskc[:], maskc_in[:])
        qscal = con.tile([128, 4], F32)
        nc.sync.dma_start(qscal[:], qscal_in[:])
        negctrK = con.tile([4, 48], F32)
        nc.sync.dma_start(negctrK[:], negctrK_in[:])
        ind4 = con.tile([4, 128], F32)
        nc.sync.dma_start(ind4[:], ind4_in[:])
        qtabA = con.tile([128, 4 * CF3H], F32)
        nc.sync.dma_start(qtabA[:], qtabA_in[:])
        gbitsA = con.tile([128, CF3H], I32)
        nc.sync.dma_start(gbitsA[:], gbitsA_in[:])
        qtabC = con.tile([128, 3 * CFNH], F32)
        nc.sync.dma_start(qtabC[:], qtabC_in[:])
        gbitsC = con.tile([128, CFNH], I32)
        nc.sync.dma_start(gbitsC[:], gbitsC_in[:])
        ident = con.tile([128, 128], F32)
        nc.sync.dma_start(ident[:], ident_in[:])
        ctr2T = con.tile([3, 128], F32)
        nc.sync.dma_start(ctr2T[:], ctr2T_in[:])
        ctrTk = con.tile([3, QH], F32)
        nc.sync.dma_start(ctrTk[:], ctrTk_in[:])
        few1c = con.tile([3, 128], F32)
        nc.sync.dma_start(few1c[:], few1c_in[:])
        feW2_b = con.tile([FE_HID, FEAT_DIM], U16)
        nc.sync.dma_start(feW2_b[:], few2_in[:])
        wbf = big.tile([128, 1665], U16)
        nc.sync.dma_start(wbf[:], wbf_in[:])
        xbf = con.tile([4, 704], U16)
        nc.sync.dma_start(xbf[:], xbf_in[:])
        biasb = con.tile([128, 26], F32)
        nc.sync.dma_start(biasb[:], bias_in[:])
        ones = con.tile([128, 1], F32)
        nc.vector.memset(ones[:], 1.0)
        izer = con.tile([128, 64], I32)
        nc.vector.memset(izer[:], 0.0)
        identb = con.tile([128, 128], BF16)
        nc.scalar.copy(identb[:], ident[:])

        # weight slices (bf16 bit views)
        Wp_fb = wbf[:, 0:128].bitcast(BF16)
        W0_b = [wbf[:, 128 + 128 * i:256 + 128 * i].bitcast(BF16)
                for i in range(NUM_BLOCKS)]
        W1_b = [wbf[:, 640 + 128 * i:768 + 128 * i].bitcast(BF16)
                for i in range(NUM_BLOCKS)]
        Wc_fb = [wbf[:, 1152 + 128 * i:1280 + 128 * i].bitcast(BF16)
                 for i in range(NUM_BLOCKS)]
        Wo_b = wbf[:, 1664:1665].bitcast(BF16)
        Wp_xb = xbf[:, 0:128].bitcast(BF16)       # 4 rows (incl. bp row)
        Wc_xb = [xbf[:, 128 + 128 * i:256 + 128 * i].bitcast(BF16)
                 for i in range(NUM_BLOCKS)]      # 4 rows (row 3 zero)
        feW1bb = xbf[0:3, 640:704].bitcast(BF16)
        feW2_t = feW2_b[:].bitcast(BF16)
        # bias cols: 1-4 g0, 5-8 hb1, 9-12 g1, 13-16 comb1,
        # 21 gO, 22 rfb, 23 bo, 24 feb1, 25 feb2
        g0_t = [biasb[:, 1 + i:2 + i] for i in range(NUM_BLOCKS)]
        hb1_t = [biasb[:, 5 + i:6 + i] for i in range(NUM_BLOCKS)]
        g1_t = [biasb[:, 9 + i:10 + i] for i in range(NUM_BLOCKS)]
        comb1_t = [biasb[:, 13 + i:14 + i] for i in range(NUM_BLOCKS)]
        gO_t = biasb[:, 21:22]
        rfb_t = biasb[:, 22:23]
        bo_t = biasb[:, 23:24]
        feb1_t = biasb[0:FE_HID, 24:25]
        feb2_t = biasb[:, 25:26]

        # ---------- windowed packed-knn helper ----------
        # qtab fields: [tc_d0 | tc_d1 (| tc_d2) | nc2], each `w` wide;
        # chain v = sum_d tc_d * q_d + nc2 = -d^2 + |q|^2 (per-row shift),
        # then pack the global candidate index into the low 12 mantissa
        # bits so MAX8/MATCH_REPLACE rounds select exact top-k + index.
        def wknn_pack(qtab_t, gbits_t, ndim, w, tagv):
            acc = None
            for dmi in range(ndim):
                t = sc.tile([128, w], F32, tag=f"{tagv}uv")
                nc.vector.scalar_tensor_tensor(
                    out=t[:], in0=qtab_t[:, dmi * w:(dmi + 1) * w],
                    scalar=qscal[:, dmi:dmi + 1],
                    in1=(qtab_t[:, ndim * w:(ndim + 1) * w] if dmi == 0
                         else acc[:]),
                    op0=OP.mult, op1=OP.add)
                acc = t
            pk = big.tile([128, w], F32, tag=f"{tagv}pk", name=f"{tagv}pk")
            nc.vector.scalar_tensor_tensor(
                out=pk[:].bitcast(I32), in0=acc[:].bitcast(I32),
                scalar=maskc[:, 0:1], in1=gbits_t[:],
                op0=OP.bitwise_and, op1=OP.bitwise_or)
            return pk

        def merge_extract(half_tops, k, tagm):
            """[128, k] per-half packed tops -> merged [64, k] indices I32."""
            mg = con.tile([QH, 2 * k], F32, tag=f"{tagm}mg", name=f"{tagm}mg")
            nc.sync.dma_start(mg[:, 0:k], half_tops[0:QH, :])
            nc.sync.dma_start(mg[:, k:2 * k], half_tops[QH:128, :])
            rounds = k // 8
            sel = con.tile([QH, k], F32, tag=f"{tagm}sel", name=f"{tagm}sel")
            cur = mg
            for r in range(rounds):
                mx = sc.tile([QH, 8], F32, tag="mx8")
                nc.vector.max(out=mx[:], in_=cur[:])
                nc.vector.tensor_copy(sel[:, 8 * r:8 * r + 8], mx[:])
                if r < rounds - 1:
                    nxt = sc.tile([QH, 2 * k], F32, tag=f"{tagm}mr",
                                  name=f"{tagm}mr")
                    nc.vector.match_replace(
                        out=nxt[:], in_to_replace=mx[:], in_values=cur[:],
                        imm_value=NEG)
                    cur = nxt
            idx = con.tile([QH, k], I32, tag=f"{tagm}idx", name=f"{tagm}idx")
            nc.vector.scalar_tensor_tensor(
                out=idx[:], in0=sel[:].bitcast(I32), scalar=maskc[0:QH, 1:2],
                in1=izer[0:QH, 0:k], op0=OP.bitwise_and, op1=OP.bitwise_or)
            return idx

        # early independent memsets fill the startup DMA wait
        xyzT = big.tile([4, QH * K_FRAME], BF16)   # row 3 = ones (bias row)
        nc.vector.memset(xyzT[:], 1.0)
        h_acc = con.tile([FE_HID, 128], F32)
        nc.vector.memset(h_acc[:], 0.0)

        # ---------- feat knn (3D, k=16) first: idx16 unblocks EdgeConv ----------
        topsA = con.tile([128, 16], F32, tag="tA", name="tA")
        topsC = con.tile([128, 32], F32, tag="tC", name="tC")

        def rnd(cur, tops, r, last, tagv, w):
            mx = sc.tile([128, 8], F32, tag="mx8")
            nc.vector.max(out=mx[:], in_=cur[:])
            nc.vector.tensor_copy(tops[:, 8 * r:8 * r + 8], mx[:])
            if last:
                return None
            nxt = dp.tile([128, w], F32, tag=f"{tagv}mr", name=f"{tagv}mr")
            nc.vector.match_replace(out=nxt[:], in_to_replace=mx[:],
                                    in_values=cur[:], imm_value=NEG)
            return nxt

        pkA = wknn_pack(qtabA, gbitsA, 3, CF3H, "tA")
        curA = rnd(pkA, topsA, 0, False, "tA", CF3H)
        rnd(curA, topsA, 1, True, "tA", CF3H)
        idx16 = merge_extract(topsA, 16, "fA")
        itall = con.tile([128, 16], I32)

        # EdgeConv pair-iterations (emitted interleaved into early t-chunks).
        featT = con.tile([FEAT_DIM, QH], BF16)
        hc_ps = ps_sm.tile([FE_HID, 128], F32, tag="pt")
        nc.tensor.matmul(hc_ps[:], few1c[:, 0:QH], ctr2T[:], start=True, stop=False)
        nc.tensor.matmul(hc_ps[:], few1c[:, QH:128], ctr2T[:], start=False, stop=True)
        hconst = con.tile([FE_HID, 128], BF16)
        nc.scalar.copy(hconst[:], hc_ps[:])
        ident64b = identb[0:QH, 0:QH]

        def pair_iter(kk):
            it2 = sc.tile([128, 1], I32, tag="it2")
            nc.sync.dma_start(it2[0:QH, :], idx16[:, kk:kk + 1])
            nc.sync.dma_start(it2[QH:128, :], idx16[:, kk + 8:kk + 9])
            nb = sc.tile([128, 3], F32, tag="nb")
            nc.gpsimd.indirect_dma_start(
                out=nb[:], out_offset=None, in_=noisy[:],
                in_offset=bass.IndirectOffsetOnAxis(ap=it2[:, :1], axis=0))
            nbT_ps = ps_sm.tile([3, 128], F32, tag="pt")
            nc.tensor.transpose(nbT_ps[:], nb[:, 0:3], ident[:])
            nbs = sc.tile([3, 128], BF16, tag="dif")
            nc.scalar.copy(nbs[:], nbT_ps[:])
            hp = ps_sm.tile([FE_HID, 128], F32, tag="pt")
            nc.tensor.matmul(hp[:], feW1bb, nbs[:], start=True, stop=False)
            nc.tensor.matmul(hp[:], ident64b, hconst[:], start=False, stop=True)
            nc.vector.scalar_tensor_tensor(
                out=h_acc[:], in0=hp[:], scalar=feb1_t, in1=h_acc[:],
                op0=OP.add, op1=OP.max)

        def feat_finish():
            hq = con.tile([FE_HID, QH], BF16)
            nc.vector.tensor_tensor(out=hq[:], in0=h_acc[:, 0:QH],
                                    in1=h_acc[:, QH:128], op=OP.max)
            fps = ps_sm.tile([FEAT_DIM, QH], F32, tag="pt")
            nc.tensor.matmul(fps[:], feW2_t, hq[:], start=True, stop=True)
            nc.scalar.activation(featT[:], fps[:], AF.Relu, bias=feb2_t,
                                 scale=1.0)

        # ---------- per-t: gather ntab; pruned clean knn -> gt ----------
        gt = con.tile([128, 16], F32)
        gp_sb = con.tile([128, 16], F32)
        lacc = con.tile([128, 1], F32)
        fsp_t = [None] * 16

        def t_prep(t):
            fsp = pre.tile([128, NTAB_W], F32, tag="fsp")
            nc.gpsimd.indirect_dma_start(
                out=fsp[:], out_offset=None, in_=ntab[:],
                in_offset=bass.IndirectOffsetOnAxis(ap=itall[:, t:t + 1], axis=0))
            fsp_t[t] = fsp
            pt1 = ps_sm.tile([3, 128], F32, tag="pt")
            nc.tensor.transpose(pt1[:], fsp[:, 0:3], ident[:])
            # centering folded in as an accumulating matmul: keeps the
            # xyzT path off the Vector queue entirely
            nc.tensor.matmul(pt1[:], negctrK[:, 3 * t:3 * t + 3], ind4[:],
                             start=False, stop=True, skip_group_check=True)
            nc.scalar.copy(xyzT[0:3, 128 * t:128 * t + 128], pt1[:])

        def t_chunk(t):
            fsp = fsp_t[t]
            tcx = fsp[:, 4:4 + CF]
            tcy = fsp[:, 4 + CF:4 + 2 * CF]
            nc2 = fsp[:, 4 + 2 * CF:4 + 3 * CF]
            czb = fsp[:, 4 + 3 * CF:4 + 4 * CF]
            u = sc.tile([128, CF], F32, tag="cu")
            nc.vector.scalar_tensor_tensor(
                out=u[:], in0=tcx, scalar=fsp[:, 0:1], in1=nc2,
                op0=OP.mult, op1=OP.add)
            v = sc.tile([128, CF], F32, tag="cv")
            nc.vector.scalar_tensor_tensor(
                out=v[:], in0=tcy, scalar=fsp[:, 1:2], in1=u[:],
                op0=OP.mult, op1=OP.add)
            mxD = sc.tile([128, 8], F32, tag="mx8")
            nc.vector.max(out=mxD[:], in_=v[:])
            zsum = sc.tile([128, 1], F32, tag="zsum")
            msk = sc.tile([128, CF], F32, tag="msk")
            nc.vector.scalar_tensor_tensor(
                out=msk[:], in0=v[:], scalar=mxD[:, 3:4], in1=czb,
                op0=OP.is_ge, op1=OP.mult, accum_out=zsum[:])
            nc.vector.scalar_tensor_tensor(
                out=gt[:, t:t + 1], in0=zsum[:], scalar=0.25, in1=fsp[:, 2:3],
                op0=OP.mult, op1=OP.subtract)

        def score_parts(blk):
            # Residual chain with the carry accumulated in place in PSUM.
            r0 = 512 * blk
            xyz_b = xyzT[:, r0:r0 + 512]
            feat_b = _bcast_qk(featT, 16 * blk, 16, 32)
            st = {}

            def p0():
                pA = ps_net.tile([HID, 512], F32, tag="net")
                nc.tensor.matmul(pA[:], Wp_xb, xyz_b, start=True, stop=False)
                nc.tensor.matmul(pA[:], Wp_fb, feat_b, start=False, stop=True)
                st["net"] = pA

            def pi_a(i):
                net = st["net"]
                h1 = sc.tile([HID, 512], BF16, tag="h1")
                nc.scalar.activation(h1[:], net[:], AF.Relu, bias=hb1_t[i],
                                     scale=g0_t[i])
                pB = ps_blk.tile([HID, 512], F32, tag="pblk")
                nc.tensor.matmul(pB[:], W0_b[i], h1[:], start=True, stop=True)
                st["pB"] = pB

            def pi_b(i):
                h2 = sc.tile([HID, 512], BF16, tag="h1")
                nc.scalar.activation(h2[:], st["pB"][:], AF.Relu,
                                     bias=comb1_t[i], scale=g1_t[i])
                net = st["net"]
                nc.tensor.matmul(net[:], Wc_xb[i], xyz_b,
                                 start=False, stop=False, skip_group_check=True)
                nc.tensor.matmul(net[:], Wc_fb[i], feat_b,
                                 start=False, stop=False, skip_group_check=True)
                nc.tensor.matmul(net[:], W1_b[i], h2[:],
                                 start=False, stop=True, skip_group_check=True)

            def p5():
                rfin = sc.tile([HID, 512], BF16, tag="h1")
                nc.scalar.activation(rfin[:], st["net"][:], AF.Relu, bias=rfb_t,
                                     scale=gO_t)
                for j in range(4):
                    t = 4 * blk + j
                    gp_ps = ps_sm.tile([128, 1], F32, tag="pt")
                    nc.tensor.matmul(gp_ps[:], rfin[:, 128 * j:128 * j + 128],
                                     Wo_b, start=True, stop=True)
                    nc.scalar.copy(gp_sb[:, t:t + 1], gp_ps[:])

            def mka(i):
                return lambda: pi_a(i)

            def mkb(i):
                return lambda: pi_b(i)

            out = [p0]
            for i in range(NUM_BLOCKS):
                out += [mka(i), mkb(i)]
            return out + [p5]

        # ---------- schedule ----------
        # frame-knn vector chain runs while the EdgeConv pair gathers
        # (emitted after, but first in the gpsimd queue) trickle in.
        pkC = wknn_pack(qtabC, gbitsC, 2, CFNH, "tC")
        curC = rnd(pkC, topsC, 0, False, "tC", CFNH)
        curC = rnd(curC, topsC, 1, False, "tC", CFNH)
        curC = rnd(curC, topsC, 2, False, "tC", CFNH)
        rnd(curC, topsC, 3, True, "tC", CFNH)
        nn32 = merge_extract(topsC, 32, "fC")
        for kk in range(8):
            pair_iter(kk)
        feat_finish()
        for _t in range(16):
            nc.sync.dma_start(itall[:, _t:_t + 1], nn32[4 * _t:4 * _t + 4, :])

        # t-loop: gather-paced preps + clean-knn chunks; score-net chunk
        # pairs (0,1) and (2,3) emitted 2-way interleaved once their xyzT
        # slices are available.
        parts = [score_parts(b) for b in range(4)]

        def emit_pair(a, b):
            for fa, fb in zip(parts[a], parts[b]):
                fa()
                fb()

        for t in range(16):
            t_prep(t)
        for t in range(16):
            t_chunk(t)
            if t == 7:
                emit_pair(0, 1)
        emit_pair(2, 3)

        # deferred loss: diff = (gp + bo) - gt; lacc = sum_t diff^2
        dtile = con.tile([128, 16], F32)
        nc.vector.scalar_tensor_tensor(
            out=dtile[:], in0=gp_sb[:], scalar=bo_t, in1=gt[:],
            op0=OP.add, op1=OP.subtract)
        nc.vector.scalar_tensor_tensor(
            out=gp_sb[:], in0=dtile[:], scalar=1.0, in1=dtile[:],
            op0=OP.mult, op1=OP.mult, accum_out=lacc[:])

        lps = ps_sm.tile([1, 1], F32, tag="pt")
        nc.tensor.matmul(lps[:], lacc[:], ones[:], start=True, stop=True)
        lsb = con.tile([1, 1], F32)
        nc.scalar.copy(lsb[:], lps[:])
        nc.sync.dma_start(partial[:], lsb[:])

    _split_multi_waits(nc)
    return nc


# ---------------- host-side preprocessing ----------------

def _bf16_rne(x):
    x = np.asarray(x, np.float32)
    u = x.view(np.uint32)
    r = ((u >> 16) + ((u >> 15) & 1)).astype(np.uint32) << 16
    out = r.view(np.float32).copy()
    return out


def _split3(x):
    x = np.asarray(x, np.float32)
    h = _bf16_rne(x)
    m = _bf16_rne(x - h)
    l = _bf16_rne(x - h - m)
    return h, m, l


def _f32_to_bf16_bits(x):
    x = np.ascontiguousarray(np.asarray(x, np.float32))
    u = x.view(np.uint32)
    r = ((u >> 16) + ((u >> 15) & 1)).astype(np.uint32)
    return (r & 0xFFFF).astype(np.uint16)


# ---- k-d tree candidate windows (index build + box lookups only) ----

def _build_kd_leaves(xy):
    """Split on the widest axis at the median down to <= LEAF points.
    Returns (leaf lo/hi bounds, leaf point lists, anc split-rects +
    density for the local radius estimate)."""
    dim = xy.shape[1]
    leaf_lo, leaf_hi, leaf_pts = [], [], []
    anc_rect, anc_rho = [], []

    def rec(ids, rect):
        lo = xy[ids].min(0)
        hi = xy[ids].max(0)
        n = len(ids)
        if ANC <= n < 2 * ANC:
            vol = max(np.prod(hi - lo), 1e-12)
            anc_rect.append(rect.copy())
            anc_rho.append(n / vol)
        if n <= LEAF:
            leaf_lo.append(lo)
            leaf_hi.append(hi)
            leaf_pts.append(ids)
            return
        ax = int(np.argmax(hi - lo))
        order = ids[np.argsort(xy[ids, ax], kind="stable")]
        h = n // 2
        cut = 0.5 * (xy[order[h - 1], ax] + xy[order[h], ax])
        r1 = rect.copy()
        r1[1, ax] = cut
        r2 = rect.copy()
        r2[0, ax] = cut
        rec(order[:h], r1)
        rec(order[h:], r2)

    rect0 = np.array([[-1e9] * dim, [1e9] * dim])
    rec(np.arange(len(xy)), rect0)
    return (np.array(leaf_lo), np.array(leaf_hi), leaf_pts,
            np.array(anc_rect), np.array(anc_rho))


def _windows(ref_xy, pts_xy, k, kappa, nmin, cf):
    """Candidate window (<= cf ref indices) per point, vectorized.
    Box query on k-d leaves; radius from the local density estimate."""
    dim = ref_xy.shape[1]
    llo, lhi, leaf_pts, arect, arho = _build_kd_leaves(ref_xy)
    npts = len(pts_xy)
    P = pts_xy[:, None, :]   # (n,1,dim)
    inrect = np.all((arect[None, :, 0] <= P) & (P < arect[None, :, 1]),
                    axis=2)
    ai = np.argmax(inrect, axis=1)
    rho = np.where(inrect.any(1), arho[ai], arho.mean())
    if dim == 2:
        R = kappa * np.sqrt(k / (np.pi * rho))
    else:
        R = kappa * (k / ((4.0 / 3.0) * np.pi * rho)) ** (1.0 / 3.0)
    lsizes = np.array([len(p) for p in leaf_pts])

    def counts_for(Rv):
        hit = np.all((llo[None] <= P + Rv[:, None, None])
                     & (lhi[None] >= P - Rv[:, None, None]), axis=2)
        return hit, hit @ lsizes

    hit, cnt = counts_for(R)
    for _ in range(6):
        low = cnt < nmin
        if not low.any():
            break
        R = np.where(low, R * 1.6, R)
        hit2, cnt2 = counts_for(R)
        hit = np.where(low[:, None], hit2, hit)
        cnt = np.where(low, cnt2, cnt)
    over = cnt > cf
    if over.any():
        lo = np.zeros(npts)
        hi = R.copy()
        best_hit = hit.copy()
        for _ in range(9):
            mid = np.where(over, 0.5 * (lo + hi), R)
            hm, cm = counts_for(mid)
            ok = cm <= cf
            upd = over & ok
            best_hit = np.where(upd[:, None], hm, best_hit)
            lo = np.where(upd, mid, lo)
            hi = np.where(over & ~ok, mid, hi)
        hit = best_hit
    out = []
    for i in range(npts):
        ids = np.concatenate([leaf_pts[j] for j in np.nonzero(hit[i])[0]]) \
            if hit[i].any() else np.zeros(0, np.int64)
        out.append(ids[:cf])
    return out


def _query_knn_tables(pn, q, dims, half_w, k, kappa, nmin):
    """Per-core dup-half candidate tables for a device-side windowed knn.
    Returns qtab [128, (ndim+1)*half_w] f32 and gbits [128, half_w] i32.
    Partition p = query (p % 64) , half (p // 64)."""
    ndim = len(dims)
    wins = _windows(pn[:, dims].astype(np.float64),
                    q[:, dims].astype(np.float64),
                    k, kappa, nmin, 2 * half_w)
    qtab = np.zeros((128, (ndim + 1) * half_w), np.float32)
    qtab[:, ndim * half_w:(ndim + 1) * half_w] = -1.0e30   # nc2 sentinel
    gbits = np.zeros((128, half_w), np.int32)
    for qi in range(QH):
        ids = np.asarray(wins[qi], np.int64)
        for h in range(2):
            part = ids[h * half_w:(h + 1) * half_w]
            m = len(part)
            if m == 0:
                continue
            p = 64 * h + qi
            c = pn[part][:, dims].astype(np.float64)
            q2 = (q[qi][list(dims)].astype(np.float64) ** 2).sum()
            for di in range(ndim):
                qtab[p, di * half_w:di * half_w + m] = 2.0 * c[:, di]
            # fixed -d^2 - 1 packing value: fold -(|q|^2+1) per partition
            # so the exponent stays ~1.0 and the 12-bit index theft
            # cannot disturb the distance ordering materially
            qtab[p, ndim * half_w:ndim * half_w + m] = (
                -(c ** 2).sum(1) - (q2 + 1.0)).astype(np.float32)
            gbits[p, :m] = part
    return qtab, gbits


def build_in_maps(inputs):
    pcl_noisy = np.ascontiguousarray(np.asarray(inputs["pcl_noisy"], np.float32))
    pcl_clean = np.ascontiguousarray(np.asarray(inputs["pcl_clean"], np.float32))
    pnt_idx = np.asarray(inputs["pnt_idx"]).astype(np.int64)

    common = {"ident": np.eye(128, dtype=np.float32),
              "maskc": np.tile(np.array([[~0xFFF, 0xFFF]], np.int32), (128, 1))}
    W = {k: np.asarray(inputs[k], np.float32) for k in
         ("feW1", "feb1", "feW2", "feb2", "Wp", "bp", "g0", "b0", "W0",
          "bf0", "g1", "b1", "W1", "bf1", "Wc", "bc", "gO", "bO", "Wo", "bo")}
    few1c = np.zeros((3, 128), np.float32)
    few1c[:, 0:FE_HID] = W["feW1"][0:3, :]
    few1c[:, FE_HID:128] = -W["feW1"][3:6, :]
    common["few1c"] = few1c
    common["few2b"] = _f32_to_bf16_bits(W["feW2"])
    wbf = np.zeros((128, 1665), np.float32)
    wbf[:, 0:128] = W["Wp"][3:, :]
    for i in range(NUM_BLOCKS):
        wbf[:, 128 + 128 * i:256 + 128 * i] = W["W0"][i]
        wbf[:, 640 + 128 * i:768 + 128 * i] = W["W1"][i]
        wbf[:, 1152 + 128 * i:1280 + 128 * i] = W["Wc"][i, 3:, :]
    wbf[:, 1664] = W["Wo"][:, 0]
    common["wbf"] = _f32_to_bf16_bits(wbf)
    xbf = np.zeros((4, 704), np.float32)
    xbf[0:3, 0:128] = W["Wp"][0:3, :]
    xbf[3, 0:128] = W["bp"]
    for i in range(NUM_BLOCKS):
        xbf[0:3, 128 + 128 * i:256 + 128 * i] = W["Wc"][i, 0:3, :]
    xbf[0:3, 640:704] = W["feW1"][3:6, :]
    common["xbf"] = _f32_to_bf16_bits(xbf)
    comb2 = [W["bf1"][i] + W["bc"][i] for i in range(NUM_BLOCKS)]
    # cumulative carry correction: PSUM net excludes all comb2 terms
    Kcum = [np.zeros(HID, np.float32)]
    for i in range(NUM_BLOCKS):
        Kcum.append(Kcum[-1] + comb2[i])
    bb = np.zeros((128, 26), np.float32)
    for i in range(NUM_BLOCKS):
        bb[:, 1 + i] = W["g0"][i]
        bb[:, 5 + i] = W["b0"][i] + W["g0"][i] * Kcum[i]
        bb[:, 9 + i] = W["g1"][i]
        bb[:, 13 + i] = W["g1"][i] * W["bf0"][i] + W["b1"][i]
    bb[:, 21] = W["gO"]
    bb[:, 22] = W["bO"] + W["gO"] * Kcum[NUM_BLOCKS]
    bb[:, 23] = W["bo"][0]
    bb[0:FE_HID, 24] = W["feb1"]
    bb[:, 25] = W["feb2"]
    common["biasb"] = bb

    # per-cloud ntab (candidate windows are query-independent) and
    # per-cloud query knn tables (both core-halves computed together)
    ntabs = []
    qknn = []
    for b in range(B):
        pn, pc = pcl_noisy[b], pcl_clean[b]
        q128 = pn[pnt_idx]
        tabs = {}
        for h in range(2):
            qh = q128[64 * h:64 * h + 64]
            tabs[h] = (_query_knn_tables(pn, qh, (0, 1, 2), CF3H, 17, 1.6, 48),
                       _query_knn_tables(pn, qh, (0, 1), CFNH, 33, 1.8, 64))
        qknn.append(tabs)
        wins = _windows(pc[:, 0:2].astype(np.float64),
                        pn[:, 0:2].astype(np.float64),
                        4, KAPPA, NMIN, CF)
        nt = np.zeros((N_NOISY, NTAB_W), np.float32)
        nt[:, 0:3] = pn
        # padded index matrix; sentinel slots use index -1 + invalid mask
        idxm = np.full((N_NOISY, CF), -1, np.int64)
        for i in range(N_NOISY):
            ids = wins[i]
            idxm[i, :len(ids)] = ids
        valid = idxm >= 0
        cl = np.clip(idxm, 0, None)
        cx, cy, cz = pc[cl, 0], pc[cl, 1], pc[cl, 2]
        nc2v = -(cx.astype(np.float64) ** 2
                 + cy.astype(np.float64) ** 2).astype(np.float32)
        nt[:, 4:4 + CF] = np.where(valid, 2.0 * cx, 0.0)
        nt[:, 4 + CF:4 + 2 * CF] = np.where(valid, 2.0 * cy, 0.0)
        nt[:, 4 + 2 * CF:4 + 3 * CF] = np.where(valid, nc2v, -1.0e30)
        nt[:, 4 + 3 * CF:4 + 4 * CF] = np.where(valid, cz, 0.0)
        ntabs.append(nt)

    in_maps = []
    for core in range(8):
        b, h = core // 2, core % 2
        m = dict(common)
        pn = pcl_noisy[b]                       # (4096, 3)
        q = pn[pnt_idx[64 * h:64 * h + 64]]     # (64, 3)

        m["noisy"] = pn
        m["ntab"] = ntabs[b]

        # windowed noisy-knn candidate tables (dup-half layout)
        (m["qtabA"], m["gbitsA"]), (m["qtabC"], m["gbitsC"]) = qknn[b][h]
        qs = np.zeros((128, 4), np.float32)
        qs[0:QH, 0:3] = q
        qs[QH:128, 0:3] = q
        m["qscal"] = qs
        nk = np.zeros((4, 48), np.float32)
        for t in range(16):
            nk[:, 3 * t:3 * t + 3] = -q[4 * t:4 * t + 4, :]
        m["negctrK"] = nk
        i4 = np.zeros((4, 128), np.float32)
        for qq in range(4):
            i4[qq, 32 * qq:32 * qq + 32] = 1.0
        m["ind4"] = i4

        m["ctrTk"] = np.ascontiguousarray(q.T)           # (3, 64)
        ctr2 = np.concatenate([q, q], axis=0)            # (128, 3)
        m["ctr2T"] = np.ascontiguousarray(ctr2.T)        # (3, 128)
        in_maps.append(m)
    return in_maps


_NC_CACHE = {}


def _get_nc():
    if "nc" not in _NC_CACHE:
        _NC_CACHE["nc"] = _build()
    return _NC_CACHE["nc"]


def kernel(**inputs) -> np.ndarray:
    from concourse.bass_utils import run_bass_kernel_spmd

    in_maps = build_in_maps(inputs)
    res = run_bass_kernel_spmd(_get_nc(), in_maps, core_ids=list(range(8)))
    total = float(np.sum([np.asarray(res.results[i]["partial"]).reshape(())
                          for i in range(8)]))
    loss = 0.5 * total / (B * N_TRAIN * K_FRAME) / DSM_SIGMA
    return np.float32(loss)
